# revision 34
# baseline (speedup 1.0000x reference)
"""DDI regularizer loss kernel for 8 Trainium2 NeuronCores.

reference semantics:
    b = (ddi > 0); S = max(b, b.T) with zero diagonal; U = triu(S, k=1)
    normalizer = max(U.sum(), 1.0)
    xu = drug_probs @ U; penalties = sum(xu * drug_probs, axis=1) / normalizer
    return penalties.mean()

Identity used here:
    mean_i(x_i^T U x_i) = <U, X^T X> / B
so the kernel computes pieces of G = X^T X (contraction over the batch is the
natural PE layout, fp8-e5m2 DoubleRow at ~155 TF/s/core), masks each piece
with U's cells (built on device from staged ddi slices) and reduces; the host
combines 8 tiny per-core vectors into the final scalar.

Work assignment (the big win over the 46.7us baseline): since G is symmetric,
a piece (lhs row-block L, rhs col-range I) covers the unordered 128x128
pair-blocks {L} x I from either orientation.  A SAT solver found an exact
cover of all 136 upper pair-blocks by 8 cores x 6 slots of widths
[512,512,512,512,384,256] anchored in one shared 512-col rhs window per core
(SPMD: one program, all per-core variation lives in the host-packed data).
That is 1134ns of matmul per chunk vs 1296 for the baseline's 6x512 slots,
with the same 1280-col X stream.  Triangular boundaries and duplicate
coverage are handled entirely by host-zeroing cells of the staged ddi slices
(no on-device iota/select), and ddi rides the wire as fp8 (sign-preserving
for the >0 test) halving its mid-stream DMA cost, upcast to bf16 on the
scalar engine.

Pipeline details (all measured off NTFF traces):
  - full-width N=512 warmup matmuls bridge from engine boot to chunk-0
    arrival so the PE activity monitor reaches full clock before real work
    (N=64 warmups were too light: first chunks ran at half rate).
  - X chunks trigger first on both HWDGE queues (chunk 0 split across both
    queues to land earlier); ddi triggers are forced mid-stream by a tiny
    DVE poison write gated on chunk 3 (scheduler wait hints get hoisted).
  - masks are (max(A, B^T) > 0) via one DVE tensor_tensor + tensor_scalar
    per slot; normalizer is one scalar-engine copy-with-accum over them.
  - the last NKTAIL chunks run slot-major so each slot's PSUM accumulation
    closes early and its fused masked-reduce (scalar_tensor_tensor with
    accum_out) overlaps the remaining slots' matmuls.
"""

import sys

for _p in ("/opt/trn_rl_repo", "/root/.axon_site/_ro/trn_rl_repo"):
    if _p not in sys.path:
        sys.path.insert(0, _p)

import numpy as np
import ml_dtypes

B, D = 4096, 2048
NBLK = 128  # lhs row-block width
NCOL = 512  # rhs col-block width
NSLOT = 6  # tile slots per core
NWARM = 5  # PE clock warmup matmuls (full-width N=512 bridge)
NKTAIL = 3  # trailing chunks run slot-major so per-slot reduces stagger
NK = B // 256  # two 128-row chunks per DoubleRow matmul

# Uniform slot template: 4x 512-wide + 1x 384-wide + 1x 256-wide matmuls per
# chunk (1134ns/chunk at fp8-DR peak vs 1296 for 6x512).  Since G is
# symmetric, a slot piece (row-block L, col range I) may cover the unordered
# pair-blocks {L} x I from EITHER orientation; the assignment below (found
# with a SAT solver) covers all 136 upper pair-blocks, duplicates are
# host-zeroed in the staged ddi slices.
SLOT_W = [512, 512, 512, 512, 384, 256]  # slot widths (cols)
SLOT_O = [0, 0, 0, 0, 128, 256]  # rhs offset within the core's R window
# per core: (anchor_block r, [row-block of each slot]); R = cols [128r, 128r+512)
Z3_ANCHORS = [0, 0, 0, 4, 4, 8, 8, 12]
Z3_ROWS = [
    [6, 15, 5, 0, 2, 0],
    [11, 9, 14, 8, 1, 9],
    [13, 7, 12, 4, 10, 3],
    [4, 9, 5, 10, 13, 14],
    [6, 11, 12, 7, 8, 15],
    [15, 14, 13, 12, 1, 0],
    [4, 10, 9, 8, 11, 0],
    [15, 14, 13, 12, 4, 5],
]

NIN = NCOL + NBLK * NSLOT  # 1280 columns in the merged X input
NDDI = sum(SLOT_W)  # 2688 mask columns per core

_CACHE = {}


def _build():
    import concourse.bass as bass
    import concourse.mybir as mybir
    from concourse import bacc
    from concourse.tile import TileContext

    f32 = mybir.dt.float32
    bf16 = mybir.dt.bfloat16
    fp8 = mybir.dt.float8e5
    op = mybir.AluOpType

    nc = bacc.Bacc("TRN2", target_bir_lowering=False, debug=False, num_devices=8)

    # xin: chunk-major DoubleRow layout - row 128k+p holds the two batch rows
    # 256k+p and 256k+128+p back to back (2560 contiguous bytes / partition).
    xin_d = nc.dram_tensor("xin", [NK * 128, 2 * NIN], fp8, kind="ExternalInput")
    # ddi rides the wire as fp8e5 (sign-preserving for the >0 test except a
    # ~1e-5 fraction of near-zero magnitudes) and is cast to bf16 by the
    # scalar engine; halves the mid-stream DMA insert on both queues.
    # Sub-diagonal cells of diagonal-straddling slots are host-zeroed in BOTH
    # ddiA and ddiB, so no on-device triangular selector is needed.
    ddiA_d = nc.dram_tensor("ddiA", [NBLK, NDDI], fp8, kind="ExternalInput")
    ddiB_d = nc.dram_tensor("ddiB", [NBLK, NDDI], fp8, kind="ExternalInput")
    out_d = nc.dram_tensor("out", [128, NSLOT + 1], f32, kind="ExternalOutput")

    with TileContext(nc) as tc:
        with (
            tc.tile_pool(name="const", bufs=1) as cpool,
            tc.tile_pool(name="io", bufs=16) as iopool,
            tc.tile_pool(name="psum", bufs=1, space="PSUM") as ppool,
            tc.tile_pool(name="tpp", bufs=1, space="PSUM") as tppool,
            tc.tile_pool(name="scr", bufs=8) as spool,
            tc.tile_pool(name="junk", bufs=2) as jpool,
        ):
            # --- gpsimd: tiny warmup-source memset (lhsT slice reuses the
            # rhs region, so only 512 columns need initializing) ---
            wsrc = cpool.tile([128, 2, NCOL], fp8, tag="wsrc")
            nc.gpsimd.memset(wsrc, 0.0)

            # --- PE HAM clock warmup: full-width N=512 matmuls keep the PE
            # at high activity from engine boot until real chunks land, so
            # the activity monitor reaches full clock before chunk 0 (N=64
            # warmups measured too light: chunks 0-2 still ran at half rate)
            wps = tppool.tile([128, NCOL], f32, tag="tp", name="warm")
            for w in range(NWARM):
                nc.tensor.matmul(
                    out=wps,
                    lhsT=wsrc[:, :, 0:NBLK],
                    rhs=wsrc[:, :, 0:NCOL],
                    start=True,
                    stop=True,
                    perf_mode=mybir.MatmulPerfMode.DoubleRow,
                )

            # --- X stream triggers first on both HWDGE queues, ddi pushed
            # behind them with an explicit scheduler wait hint ---
            xts = []
            xin_ap = xin_d.ap().rearrange("(k p) (i c) -> k p i c", p=128, i=2)
            for k in range(NK):
                xt = iopool.tile([128, 2, NIN], fp8, tag="xt")
                if k <= 1:
                    # first two chunks split across both queues so the PE
                    # (running from ~10.8us after a short warmup bridge)
                    # is never starved while the stream ramps
                    h = NIN // 2
                    nc.sync.dma_start(out=xt[:, :, 0:h], in_=xin_ap[k][:, :, 0:h])
                    nc.scalar.dma_start(out=xt[:, :, h:], in_=xin_ap[k][:, :, h:])
                else:
                    eng = nc.sync if k % 2 == 0 else nc.scalar
                    eng.dma_start(out=xt, in_=xin_ap[k])
                xts.append(xt)

            # ddi loads must ride the queues mid-stream: a tiny DVE poison
            # write into each dest tile, gated on chunk 3's arrival, forces
            # the triggers behind the early X chunks (wait hints alone get
            # hoisted; gating on a later chunk pushes them behind ALL chunks)
            ddiA_8 = cpool.tile([NBLK, NDDI], fp8, tag="ddiA8")
            ddiBT_8 = cpool.tile([NBLK, NDDI], fp8, tag="ddiBT8")
            nc.vector.tensor_scalar(
                out=ddiA_8[:, 0:1], in0=xts[3][:, 0, 0:1],
                scalar1=0.0, scalar2=None, op0=op.mult,
            )
            nc.vector.tensor_scalar(
                out=ddiBT_8[:, 0:1], in0=xts[3][:, 0, 0:1],
                scalar1=0.0, scalar2=None, op0=op.mult,
            )
            nc.sync.dma_start(out=ddiA_8, in_=ddiA_d.ap())
            nc.scalar.dma_start(out=ddiBT_8, in_=ddiB_d.ap())
            # scalar-engine upcast fp8 -> bf16 (DVE's fp8 path is the slow one)
            ddiA_sb = cpool.tile([NBLK, NDDI], bf16, tag="ddiA")
            ddiBT_raw = cpool.tile([NBLK, NDDI], bf16, tag="ddiBTr")
            nc.scalar.activation(
                out=ddiA_sb, in_=ddiA_8, func=mybir.ActivationFunctionType.Copy,
            )
            nc.scalar.activation(
                out=ddiBT_raw, in_=ddiBT_8, func=mybir.ActivationFunctionType.Copy,
            )

            # --- G tiles: accumulating matmuls, k-outer so the X stream is
            # consumed strictly in order; per-slot PSUM tiles so the final
            # per-slot reduces can stagger ---
            gps = [
                ppool.tile([128, SLOT_W[t]], f32, tag=f"gps{t}", name=f"gps{t}")
                for t in range(NSLOT)
            ]
            # chunk-outer while streaming; the last NKTAIL chunks flip to
            # slot-major so slot t's accumulation closes early and its masked
            # reduce overlaps the remaining slots' matmuls
            sched = [(k, t) for k in range(NK - NKTAIL) for t in range(NSLOT)]
            sched += [(k, t) for t in range(NSLOT) for k in range(NK - NKTAIL, NK)]
            for k, t in sched:
                xt = xts[k]
                c0 = NCOL + t * NBLK
                r0 = SLOT_O[t]
                nc.tensor.matmul(
                    out=gps[t],
                    lhsT=xt[:, :, c0 : c0 + NBLK],
                    rhs=xt[:, :, r0 : r0 + SLOT_W[t]],
                    start=(k == 0),
                    stop=(k == NK - 1),
                    perf_mode=mybir.MatmulPerfMode.DoubleRow,
                )

            # masks on DVE, overlapped with the matmul phase:
            # U_tile = (max(rawA, rawB^T) > 0); triu/duplicate-coverage
            # handled entirely by host zeroing of the staged ddi slices
            out_sb = cpool.tile([128, NSLOT + 1], f32, tag="out")
            maskc = cpool.tile([128, NDDI], bf16, tag="maskc")
            doff = [0]
            for w in SLOT_W:
                doff.append(doff[-1] + w)
            for t in range(NSLOT):
                lo, hi = doff[t], doff[t + 1]
                mraw = spool.tile([128, SLOT_W[t]], bf16, tag="mraw")
                nc.vector.tensor_tensor(
                    out=mraw, in0=ddiA_sb[:, lo:hi], in1=ddiBT_raw[:, lo:hi],
                    op=op.max,
                )
                nc.vector.tensor_scalar(
                    out=maskc[:, lo:hi], in0=mraw, scalar1=0.0,
                    scalar2=None, op0=op.is_gt,
                )

            # normalizer sum(U): one scalar-engine accum over all masks
            mjunk = jpool.tile([128, NDDI], bf16, tag="mjunk")
            nc.scalar.activation(
                out=mjunk, in_=maskc,
                func=mybir.ActivationFunctionType.Copy,
                accum_out=out_sb[:, NSLOT : NSLOT + 1],
            )

            # masked reductions sum(G_t * mask_t): per-slot fused DVE ops with
            # accum_out, each ready right after its slot's k=15 matmul
            for t in range(NSLOT):
                gjunk = jpool.tile([128, SLOT_W[t]], f32, tag=f"gj{t % 2}")
                nc.vector.scalar_tensor_tensor(
                    out=gjunk, in0=gps[t], scalar=1.0,
                    in1=maskc[:, doff[t] : doff[t + 1]],
                    op0=op.mult, op1=op.mult,
                    accum_out=out_sb[:, t : t + 1],
                )

            nc.sync.dma_start(out=out_d.ap(), in_=out_sb)

    nc.compile()
    return nc


def _in_maps(drug_probs, ddi_matrix):
    fp8 = ml_dtypes.float8_e5m2
    xq = drug_probs.astype(fp8)
    db8 = ddi_matrix.astype(fp8)
    zero8 = np.zeros((), dtype=fp8)

    # ownership: each unordered pair of 128-blocks is kept by exactly the
    # first piece that covers it; later coverers get their ddi slice zeroed
    owner = {}
    for cidx in range(8):
        r = Z3_ANCHORS[cidx]
        for s in range(NSLOT):
            row = Z3_ROWS[cidx][s]
            lo_b = r + SLOT_O[s] // NBLK
            for b in range(lo_b, lo_b + SLOT_W[s] // NBLK):
                key = (min(row, b), max(row, b))
                owner.setdefault(key, (cidx, s, b))

    maps = []
    for cidx in range(8):
        r = Z3_ANCHORS[cidx]
        rows = Z3_ROWS[cidx]
        xin = np.concatenate(
            [xq[:, r * NBLK : r * NBLK + NCOL]]
            + [xq[:, i * NBLK : (i + 1) * NBLK] for i in rows],
            axis=1,
        )
        # chunk-major DoubleRow packing: [4096, NIN] -> [16*128, 2*NIN]
        xin = (
            xin.reshape(NK, 2, 128, NIN)
            .transpose(0, 2, 1, 3)
            .reshape(NK * 128, 2 * NIN)
        )
        p = np.arange(128)[:, None]
        ddiA_l, ddiB_l = [], []
        for s in range(NSLOT):
            row = rows[s]
            w = SLOT_W[s]
            lo = r * NBLK + SLOT_O[s]  # global col of slice start
            a = db8[row * NBLK : (row + 1) * NBLK, lo : lo + w].copy()
            bt = db8[lo : lo + w, row * NBLK : (row + 1) * NBLK].T.copy()
            for j in range(w // NBLK):
                b = lo // NBLK + j
                key = (min(row, b), max(row, b))
                sl = slice(j * NBLK, (j + 1) * NBLK)
                if owner[key] != (cidx, s, b):
                    a[:, sl] = zero8
                    bt[:, sl] = zero8
                elif b == row:
                    # self pair-block: keep only the strict upper triangle
                    keep = np.arange(NBLK)[None, :] > p
                    a[:, sl] = np.where(keep, a[:, sl], zero8)
                    bt[:, sl] = np.where(keep, bt[:, sl], zero8)
            ddiA_l.append(a)
            ddiB_l.append(bt)
        maps.append(
            {
                "xin": np.ascontiguousarray(xin),
                "ddiA": np.ascontiguousarray(np.concatenate(ddiA_l, axis=1)),
                "ddiB": np.ascontiguousarray(np.concatenate(ddiB_l, axis=1)),
            }
        )
    return maps


def kernel(drug_probs, ddi_matrix, **_run_kwargs):
    from concourse.bass_utils import run_bass_kernel_spmd

    if "nc" not in _CACHE:
        _CACHE["nc"] = _build()
    nc = _CACHE["nc"]

    maps = _in_maps(np.asarray(drug_probs), np.asarray(ddi_matrix))
    res = run_bass_kernel_spmd(nc, maps, list(range(8)), **_run_kwargs)
    _CACHE["last_result"] = res

    gsum = 0.0
    msum = 0.0
    for core_out in res.results:
        o = core_out["out"].astype(np.float64)
        gsum += o[:, 0:NSLOT].sum()
        msum += o[:, NSLOT].sum()
    normalizer = max(msum, 1.0)
    return np.asarray(gsum / (B * normalizer), dtype=np.float32)


# revision 36
# speedup vs baseline: 1.1179x; 1.1179x over previous
"""DDI regularizer loss kernel for 8 Trainium2 NeuronCores.

reference semantics:
    b = (ddi > 0); S = max(b, b.T) with zero diagonal; U = triu(S, k=1)
    normalizer = max(U.sum(), 1.0)
    xu = drug_probs @ U; penalties = sum(xu * drug_probs, axis=1) / normalizer
    return penalties.mean()

Identity used here:
    mean_i(x_i^T U x_i) = <U, X^T X> / B
so the kernel computes pieces of G = X^T X (contraction over the batch is the
natural PE layout, fp8-e5m2 DoubleRow at ~155 TF/s/core), masks each piece
with U's cells (built on device from staged ddi slices) and reduces; the host
combines 8 tiny per-core vectors into the final scalar.

Work assignment (the big win over the 46.7us baseline): since G is symmetric,
a piece (lhs row-block L, rhs col-range I) covers the unordered 128x128
pair-blocks {L} x I from either orientation.  A SAT solver found an exact
cover of all 136 upper pair-blocks by 8 cores x 6 slots of widths
[512,512,512,512,384,256] anchored in one shared 512-col rhs window per core
(SPMD: one program, all per-core variation lives in the host-packed data).
That is 1134ns of matmul per chunk vs 1296 for the baseline's 6x512 slots,
with the same 1280-col X stream.  Triangular boundaries and duplicate
coverage are handled entirely by host-zeroing cells of the staged ddi slices
(no on-device iota/select), and ddi rides the wire as fp8 (sign-preserving
for the >0 test) halving its mid-stream DMA cost, upcast to bf16 on the
scalar engine.

Pipeline details (all measured off NTFF traces):
  - full-width N=512 warmup matmuls bridge from engine boot to chunk-0
    arrival so the PE activity monitor reaches full clock before real work
    (N=64 warmups were too light: first chunks ran at half rate).
  - X chunks trigger first on both HWDGE queues (chunk 0 split across both
    queues to land earlier); ddi triggers are forced mid-stream by a tiny
    DVE poison write gated on chunk 3 (scheduler wait hints get hoisted).
  - masks are (max(A, B^T) > 0) via one DVE tensor_tensor + tensor_scalar
    per slot; normalizer is one scalar-engine copy-with-accum over them.
  - the last NKTAIL chunks run slot-major so each slot's PSUM accumulation
    closes early and its fused masked-reduce (scalar_tensor_tensor with
    accum_out) overlaps the remaining slots' matmuls.
"""

import sys

for _p in ("/opt/trn_rl_repo", "/root/.axon_site/_ro/trn_rl_repo"):
    if _p not in sys.path:
        sys.path.insert(0, _p)

import numpy as np
import ml_dtypes

B, D = 4096, 2048
NBLK = 128  # lhs row-block width
NCOL = 512  # rhs col-block width
NSLOT = 6  # tile slots per core
NWARM = 8  # PE clock warmup matmuls (full-width N=512 bridge)
NKTAIL = 3  # trailing chunks run slot-major so per-slot reduces stagger
NK = B // 256  # two 128-row chunks per DoubleRow matmul

# Uniform slot template: 4x 512-wide + 1x 384-wide + 1x 256-wide matmuls per
# chunk (1134ns/chunk at fp8-DR peak vs 1296 for 6x512).  Since G is
# symmetric, a slot piece (row-block L, col range I) may cover the unordered
# pair-blocks {L} x I from EITHER orientation; the assignment below (found
# with a SAT solver) covers all 136 upper pair-blocks, duplicates are
# host-zeroed in the staged ddi slices.
SLOT_W = [512, 512, 512, 512, 384, 256]  # slot widths (cols)
SLOT_O = [0, 0, 0, 0, 128, 256]  # rhs offset within the core's R window
# per core: (anchor_block r, [row-block of each slot]); R = cols [128r, 128r+512)
Z3_ANCHORS = [0, 0, 0, 4, 4, 8, 8, 12]
Z3_ROWS = [
    [6, 15, 5, 0, 2, 0],
    [11, 9, 14, 8, 1, 9],
    [13, 7, 12, 4, 10, 3],
    [4, 9, 5, 10, 13, 14],
    [6, 11, 12, 7, 8, 15],
    [15, 14, 13, 12, 1, 0],
    [4, 10, 9, 8, 11, 0],
    [15, 14, 13, 12, 4, 5],
]

NIN = NCOL + NBLK * NSLOT  # 1280 columns in the merged X input
NDDI = sum(SLOT_W)  # 2688 mask columns per core

_CACHE = {}


def _build():
    import concourse.bass as bass
    import concourse.mybir as mybir
    from concourse import bacc
    from concourse.tile import TileContext

    f32 = mybir.dt.float32
    bf16 = mybir.dt.bfloat16
    fp8 = mybir.dt.float8e5
    op = mybir.AluOpType

    nc = bacc.Bacc("TRN2", target_bir_lowering=False, debug=False, num_devices=8)

    # xin: chunk-major DoubleRow layout - row 128k+p holds the two batch rows
    # 256k+p and 256k+128+p back to back (2560 contiguous bytes / partition).
    xin_d = nc.dram_tensor("xin", [NK * 128, 2 * NIN], fp8, kind="ExternalInput")
    # ddi rides the wire as fp8e5 (sign-preserving for the >0 test except a
    # ~1e-5 fraction of near-zero magnitudes) and is cast to bf16 by the
    # scalar engine; halves the mid-stream DMA insert on both queues.
    # Sub-diagonal cells of diagonal-straddling slots are host-zeroed in BOTH
    # ddiA and ddiB, so no on-device triangular selector is needed.
    ddiA_d = nc.dram_tensor("ddiA", [NBLK, NDDI], fp8, kind="ExternalInput")
    ddiB_d = nc.dram_tensor("ddiB", [NBLK, NDDI], fp8, kind="ExternalInput")
    out_d = nc.dram_tensor("out", [128, NSLOT + 1], f32, kind="ExternalOutput")

    with TileContext(nc) as tc:
        with (
            tc.tile_pool(name="const", bufs=1) as cpool,
            tc.tile_pool(name="io", bufs=16) as iopool,
            tc.tile_pool(name="psum", bufs=1, space="PSUM") as ppool,
            tc.tile_pool(name="tpp", bufs=1, space="PSUM") as tppool,
            tc.tile_pool(name="scr", bufs=8) as spool,
            tc.tile_pool(name="junk", bufs=2) as jpool,
        ):
            # --- gpsimd: tiny warmup-source memset (lhsT slice reuses the
            # rhs region, so only 512 columns need initializing) ---
            wsrc = cpool.tile([128, 2, NCOL], fp8, tag="wsrc")
            nc.gpsimd.memset(wsrc, 0.0)

            # --- PE HAM clock warmup: full-width N=512 matmuls keep the PE
            # at high activity from engine boot until real chunks land, so
            # the activity monitor reaches full clock before chunk 0 (N=64
            # warmups measured too light: chunks 0-2 still ran at half rate)
            wps = tppool.tile([128, NCOL], f32, tag="tp", name="warm")
            for w in range(NWARM):
                nc.tensor.matmul(
                    out=wps,
                    lhsT=wsrc[:, :, 0:NBLK],
                    rhs=wsrc[:, :, 0:NCOL],
                    start=True,
                    stop=True,
                    perf_mode=mybir.MatmulPerfMode.DoubleRow,
                )

            # --- X stream triggers first on both HWDGE queues, ddi pushed
            # behind them with an explicit scheduler wait hint ---
            xts = []
            xin_ap = xin_d.ap().rearrange("(k p) (i c) -> k p i c", p=128, i=2)
            for k in range(NK):
                xt = iopool.tile([128, 2, NIN], fp8, tag="xt")
                if k == 0:
                    # first chunk split across both queues so the PE can
                    # start ~0.8us earlier
                    h = NIN // 2
                    nc.sync.dma_start(out=xt[:, :, 0:h], in_=xin_ap[k][:, :, 0:h])
                    nc.scalar.dma_start(out=xt[:, :, h:], in_=xin_ap[k][:, :, h:])
                else:
                    eng = nc.sync if k % 2 == 0 else nc.scalar
                    eng.dma_start(out=xt, in_=xin_ap[k])
                xts.append(xt)

            # ddi loads must ride the queues mid-stream: a tiny DVE poison
            # write into each dest tile, gated on chunk 3's arrival, forces
            # the triggers behind the early X chunks (wait hints alone get
            # hoisted; gating on a later chunk pushes them behind ALL chunks)
            ddiA_8 = cpool.tile([NBLK, NDDI], fp8, tag="ddiA8")
            ddiBT_8 = cpool.tile([NBLK, NDDI], fp8, tag="ddiBT8")
            nc.vector.tensor_scalar(
                out=ddiA_8[:, 0:1], in0=xts[3][:, 0, 0:1],
                scalar1=0.0, scalar2=None, op0=op.mult,
            )
            nc.vector.tensor_scalar(
                out=ddiBT_8[:, 0:1], in0=xts[3][:, 0, 0:1],
                scalar1=0.0, scalar2=None, op0=op.mult,
            )
            nc.sync.dma_start(out=ddiA_8, in_=ddiA_d.ap())
            nc.scalar.dma_start(out=ddiBT_8, in_=ddiB_d.ap())
            # scalar-engine upcast fp8 -> bf16 (DVE's fp8 path is the slow one)
            ddiA_sb = cpool.tile([NBLK, NDDI], bf16, tag="ddiA")
            ddiBT_raw = cpool.tile([NBLK, NDDI], bf16, tag="ddiBTr")
            nc.scalar.activation(
                out=ddiA_sb, in_=ddiA_8, func=mybir.ActivationFunctionType.Copy,
            )
            nc.scalar.activation(
                out=ddiBT_raw, in_=ddiBT_8, func=mybir.ActivationFunctionType.Copy,
            )

            # --- G tiles: accumulating matmuls, k-outer so the X stream is
            # consumed strictly in order; per-slot PSUM tiles so the final
            # per-slot reduces can stagger ---
            gps = [
                ppool.tile([128, SLOT_W[t]], f32, tag=f"gps{t}", name=f"gps{t}")
                for t in range(NSLOT)
            ]
            # chunk-outer while streaming; the last NKTAIL chunks flip to
            # slot-major so slot t's accumulation closes early and its masked
            # reduce overlaps the remaining slots' matmuls
            sched = [(k, t) for k in range(NK - NKTAIL) for t in range(NSLOT)]
            sched += [(k, t) for t in range(NSLOT) for k in range(NK - NKTAIL, NK)]
            for k, t in sched:
                xt = xts[k]
                c0 = NCOL + t * NBLK
                r0 = SLOT_O[t]
                nc.tensor.matmul(
                    out=gps[t],
                    lhsT=xt[:, :, c0 : c0 + NBLK],
                    rhs=xt[:, :, r0 : r0 + SLOT_W[t]],
                    start=(k == 0),
                    stop=(k == NK - 1),
                    perf_mode=mybir.MatmulPerfMode.DoubleRow,
                )

            # masks on DVE, overlapped with the matmul phase:
            # U_tile = (max(rawA, rawB^T) > 0); triu/duplicate-coverage
            # handled entirely by host zeroing of the staged ddi slices
            out_sb = cpool.tile([128, NSLOT + 1], f32, tag="out")
            maskc = cpool.tile([128, NDDI], bf16, tag="maskc")
            doff = [0]
            for w in SLOT_W:
                doff.append(doff[-1] + w)
            for t in range(NSLOT):
                lo, hi = doff[t], doff[t + 1]
                mraw = spool.tile([128, SLOT_W[t]], bf16, tag="mraw")
                nc.vector.tensor_tensor(
                    out=mraw, in0=ddiA_sb[:, lo:hi], in1=ddiBT_raw[:, lo:hi],
                    op=op.max,
                )
                nc.vector.tensor_scalar(
                    out=maskc[:, lo:hi], in0=mraw, scalar1=0.0,
                    scalar2=None, op0=op.is_gt,
                )

            # normalizer sum(U): one scalar-engine accum over all masks
            mjunk = jpool.tile([128, NDDI], bf16, tag="mjunk")
            nc.scalar.activation(
                out=mjunk, in_=maskc,
                func=mybir.ActivationFunctionType.Copy,
                accum_out=out_sb[:, NSLOT : NSLOT + 1],
            )

            # masked reductions sum(G_t * mask_t): per-slot fused DVE ops with
            # accum_out, each ready right after its slot's k=15 matmul
            for t in range(NSLOT):
                gjunk = jpool.tile([128, SLOT_W[t]], f32, tag=f"gj{t % 2}")
                nc.vector.scalar_tensor_tensor(
                    out=gjunk, in0=gps[t], scalar=1.0,
                    in1=maskc[:, doff[t] : doff[t + 1]],
                    op0=op.mult, op1=op.mult,
                    accum_out=out_sb[:, t : t + 1],
                )

            nc.sync.dma_start(out=out_d.ap(), in_=out_sb)

    nc.compile()
    return nc


def _in_maps(drug_probs, ddi_matrix):
    fp8 = ml_dtypes.float8_e5m2
    xq = drug_probs.astype(fp8)
    db8 = ddi_matrix.astype(fp8)
    zero8 = np.zeros((), dtype=fp8)

    # ownership: each unordered pair of 128-blocks is kept by exactly the
    # first piece that covers it; later coverers get their ddi slice zeroed
    owner = {}
    for cidx in range(8):
        r = Z3_ANCHORS[cidx]
        for s in range(NSLOT):
            row = Z3_ROWS[cidx][s]
            lo_b = r + SLOT_O[s] // NBLK
            for b in range(lo_b, lo_b + SLOT_W[s] // NBLK):
                key = (min(row, b), max(row, b))
                owner.setdefault(key, (cidx, s, b))

    maps = []
    for cidx in range(8):
        r = Z3_ANCHORS[cidx]
        rows = Z3_ROWS[cidx]
        xin = np.concatenate(
            [xq[:, r * NBLK : r * NBLK + NCOL]]
            + [xq[:, i * NBLK : (i + 1) * NBLK] for i in rows],
            axis=1,
        )
        # chunk-major DoubleRow packing: [4096, NIN] -> [16*128, 2*NIN]
        xin = (
            xin.reshape(NK, 2, 128, NIN)
            .transpose(0, 2, 1, 3)
            .reshape(NK * 128, 2 * NIN)
        )
        p = np.arange(128)[:, None]
        ddiA_l, ddiB_l = [], []
        for s in range(NSLOT):
            row = rows[s]
            w = SLOT_W[s]
            lo = r * NBLK + SLOT_O[s]  # global col of slice start
            a = db8[row * NBLK : (row + 1) * NBLK, lo : lo + w].copy()
            bt = db8[lo : lo + w, row * NBLK : (row + 1) * NBLK].T.copy()
            for j in range(w // NBLK):
                b = lo // NBLK + j
                key = (min(row, b), max(row, b))
                sl = slice(j * NBLK, (j + 1) * NBLK)
                if owner[key] != (cidx, s, b):
                    a[:, sl] = zero8
                    bt[:, sl] = zero8
                elif b == row:
                    # self pair-block: keep only the strict upper triangle
                    keep = np.arange(NBLK)[None, :] > p
                    a[:, sl] = np.where(keep, a[:, sl], zero8)
                    bt[:, sl] = np.where(keep, bt[:, sl], zero8)
            ddiA_l.append(a)
            ddiB_l.append(bt)
        maps.append(
            {
                "xin": np.ascontiguousarray(xin),
                "ddiA": np.ascontiguousarray(np.concatenate(ddiA_l, axis=1)),
                "ddiB": np.ascontiguousarray(np.concatenate(ddiB_l, axis=1)),
            }
        )
    return maps


def kernel(drug_probs, ddi_matrix, **_run_kwargs):
    from concourse.bass_utils import run_bass_kernel_spmd

    if "nc" not in _CACHE:
        _CACHE["nc"] = _build()
    nc = _CACHE["nc"]

    maps = _in_maps(np.asarray(drug_probs), np.asarray(ddi_matrix))
    res = run_bass_kernel_spmd(nc, maps, list(range(8)), **_run_kwargs)
    _CACHE["last_result"] = res

    gsum = 0.0
    msum = 0.0
    for core_out in res.results:
        o = core_out["out"].astype(np.float64)
        gsum += o[:, 0:NSLOT].sum()
        msum += o[:, NSLOT].sum()
    normalizer = max(msum, 1.0)
    return np.asarray(gsum / (B * normalizer), dtype=np.float32)
